# revision 1
# baseline (speedup 1.0000x reference)
"""Block-diagonal ZF equalizer (nn_BDEqualizer) as a Trainium2 Bass kernel.

Math: for every resource element (b, s, f) and UE u, solve the 8x8 complex
system H_u x_u = y_u where H_u[i, j] = h[b, 0, 8u+i, u, j, s, f] and
y_u[i] = y[b, 0, 8u+i, s, f].  Output x as [B, 1, 32, S, F, 2] (re/im last).

Strategy (data-parallel over the fft axis, per the sharding hint):
  - 8 cores, each owns a contiguous 128-subcarrier slice of F=1024.
  - Host pre-extracts the block-diagonal channel blocks (pure indexing) and
    ships per-core shards hd[B, U, 8, 8, S, 128] / yd[B, U, 8, S, 128].
  - On-chip layout: subcarriers on the 128 SBUF partitions, the other RE
    axes (u, b-pair, s) = 112 along the free dim.  Each of the 9 augmented
    matrix columns (8 of H + rhs) is a "plane" of 8 rows; every Gaussian
    elimination step is a full-width [128, n*112] elementwise op on the
    Vector engine, with per-RE pivot reciprocals.  Unpivoted LU + Jordan
    back-substitution, complex arithmetic as separate re/im tiles.
  - TensorE transposes move between the DMA-friendly [(u,b,s), f] staging
    layout and the compute layout [f, (u,b,s)]; ScalarE drains PSUM.
  - Two chunks (b in {0,1} then {2,3}) double-buffer load against compute.
  - Elimination updates run on groups of up to 4 planes per instruction
    (the plane index rides a third AP dim: [128, w, n, 112] broadcast
    views), cutting VectorE instruction-issue overhead ~2x; pivot squares
    go to ScalarE; each solution row is stored (TensorE transpose + DMA)
    as soon as its back-substitution step finishes, hiding the store under
    the remaining VectorE back pass.  The kernel is VectorE-bound (fp32
    tensor_tensor is 1 elem/lane/cycle and the ~38M-element-op/core solve
    has no matmul structure for TensorE, while GPSIMD is locked out of the
    shared SBUF port during 2-port DVE ops).

Measured: cost-model (TimelineSim) per-core time ~496 us (~94% VectorE
busy, vs a ~420 us pure element-streaming floor); output vs the fp32 jax
reference: rel-L2 2.9e-4, abs-max 0.77 on a +-1203 output (6.4e-4
scale-relative; unpivoted GE tail on the worst-conditioned REs).
"""

import os

import numpy as np

import concourse.bacc as bacc
import concourse.mybir as mybir
from concourse.bass_utils import run_bass_kernel_spmd
from concourse.masks import make_identity
from concourse.tile import TileContext

B, NRX, NR, U, A, S, F = 4, 1, 32, 4, 8, 14, 1024
NCORES = 8
FS = F // NCORES        # 128 subcarriers per core
NB = 2                  # batch entries per chunk
NCH = B // NB           # chunks per core
M = U * NB * S          # 112 RE columns per chunk (u, b, s)
NP = 9                  # augmented planes: 8 matrix columns + rhs
F32 = mybir.dt.float32
AL = mybir.AluOpType

LAST_RESULTS = None     # BassKernelResults of the most recent run (for test.py)


def _off(j, i):
    """Free-dim offset of (plane j, row i) inside an H supertile."""
    return (j * A + i) * M


def _build():
    nc = bacc.Bacc(trn_type="TRN2")

    # Host-prepped layouts, chosen so every per-(chunk, i) DMA slice is
    # stride-collapsible: hd[i, u, b, s, j, f], yd[i, u, b, s, f],
    # out[i, u, b, s, f, c].  (i = matrix row, j = matrix column.)
    hdre = nc.dram_tensor("hd_re", [A, U, B, S, A, FS], F32, kind="ExternalInput")
    hdim = nc.dram_tensor("hd_im", [A, U, B, S, A, FS], F32, kind="ExternalInput")
    ydre = nc.dram_tensor("yd_re", [A, U, B, S, FS], F32, kind="ExternalInput")
    ydim = nc.dram_tensor("yd_im", [A, U, B, S, FS], F32, kind="ExternalInput")
    out = nc.dram_tensor("out", [A, U, B, S, FS, 2], F32, kind="ExternalOutput")

    with TileContext(nc) as tc:
        with (
            tc.tile_pool(name="consts", bufs=1) as consts,
            tc.tile_pool(name="supers", bufs=2) as supers,
            tc.tile_pool(name="work", bufs=1) as work,
            tc.tile_pool(name="stg", bufs=3) as stg,
            tc.tile_pool(name="stgo", bufs=3) as stgo,
            tc.tile_pool(name="psin", bufs=3, space="PSUM") as psin,
            tc.tile_pool(name="psy", bufs=2, space="PSUM") as psy_pool,
            tc.tile_pool(name="pso", bufs=2, space="PSUM") as pso_pool,
        ):
            ident = consts.tile([128, 128], F32)
            make_identity(nc, ident)

            for ci in range(NCH):
                b0 = ci * NB
                HRe = supers.tile([128, (NP + 1) * A * M], F32, tag="HRe")
                HIm = supers.tile([128, (NP + 1) * A * M], F32, tag="HIm")
                hsup = (HRe, HIm)

                def row(T, j, i):
                    return T[:, _off(j, i) : _off(j, i) + M]

                def rows3(T, j, i0, n):
                    base = _off(j, i0)
                    return T[:, base : base + n * M].rearrange(
                        "p (r c) -> p r c", r=n
                    )

                def bc(ap, n):
                    return ap[:, None, :].broadcast_to([128, n, M])

                # ---------------- load h ----------------
                for comp in range(2):
                    hsrc = (hdre, hdim)[comp]
                    for i in range(A):
                        stage = stg.tile([M, A * FS], F32, tag="stage")
                        src = hsrc[i, :, b0 : b0 + NB]
                        nc.sync.dma_start(stage, src)
                        for jg in range(2):
                            ps = psin.tile([128, 4 * M], F32, tag="psin")
                            for q in range(4):
                                j = jg * 4 + q
                                nc.tensor.transpose(
                                    ps[:, q * M : (q + 1) * M],
                                    stage[:, j * FS : (j + 1) * FS],
                                    ident[:M, :M],
                                )
                            base = _off(jg * 4, i)
                            dst = hsup[comp][:, base : base + 4 * A * M].rearrange(
                                "p (q c) -> p q c", q=4
                            )[:, :, :M]
                            src3 = ps.rearrange("p (q c) -> p q c", q=4)
                            nc.scalar.copy(dst, src3)

                # ---------------- load y ----------------
                for comp in range(2):
                    ysrc = (ydre, ydim)[comp]
                    for i in range(A):
                        sy = stg.tile([M, FS], F32, tag="stagey")
                        nc.sync.dma_start(sy, ysrc[i, :, b0 : b0 + NB])
                        py = psy_pool.tile([128, M], F32, tag="psy")
                        nc.tensor.transpose(py, sy, ident[:M, :M])
                        nc.scalar.copy(row(hsup[comp], 8, i), py)

                # ---------------- solve ----------------
                # INV holds the pivot reciprocals: ir block [0:A*M], ii block
                # [A*M:2*A*M], plus A*M padding so the (ir_k, ii_k) stride-
                # A*M pair view can be built by slice+rearrange for every k.
                INV = work.tile([128, 3 * A * M], F32, tag="INV")
                FRe = work.tile([128, (A - 1) * M], F32, tag="FRe")
                FIm = work.tile([128, (A - 1) * M], F32, tag="FIm")
                # PAs is the single wide product scratch for the width-4
                # elimination groups (DVE is in-order, so product->accumulate
                # can reuse one buffer); PBs only needs the factor-prep pair.
                PAs = work.tile([128, 4 * (A - 1) * M], F32, tag="PAs")
                PBs = work.tile([128, 4 * (A - 1) * M], F32, tag="PBs")
                PCs = work.tile([128, (A - 1) * M], F32, tag="PCs")
                PDs = work.tile([128, (A - 1) * M], F32, tag="PDs")
                TD = work.tile([128, M], F32, tag="TD")
                TU = work.tile([128, M], F32, tag="TU")
                TR = work.tile([128, M], F32, tag="TR")

                def sc3(T, n):
                    return T[:, : n * M].rearrange("p (r c) -> p r c", r=n)

                def sc4(T, n):
                    # [128, 2, n, M] j-major view of scratch
                    return T[:, : 2 * n * M].rearrange(
                        "p (j r c) -> p j r c", j=2, r=n
                    )

                def sc_half(T, h, n):
                    return T[:, h * n * M : (h + 1) * n * M]

                def inv_pair(k, n=None):
                    # (ir_k, ii_k) as [128, 2, M]; broadcast over n rows if set
                    v = INV[:, k * M : k * M + 2 * A * M].rearrange(
                        "p (j c) -> p j c", j=2
                    )[:, :, :M]
                    if n is None:
                        return v
                    return v[:, :, None, :].broadcast_to([128, 2, n, M])

                # forward elimination
                for k in range(A):
                    a = row(HRe, k, k)
                    b_ = row(HIm, k, k)
                    nc.scalar.square(TD, a)
                    nc.scalar.square(TU, b_)
                    nc.vector.tensor_add(TD, TD, TU)
                    nc.vector.reciprocal(TR, TD)
                    irk = INV[:, k * M : (k + 1) * M]
                    iik = INV[:, (A + k) * M : (A + k + 1) * M]
                    nc.vector.tensor_mul(irk, a, TR)
                    nc.vector.tensor_mul(iik, b_, TR)
                    n = A - 1 - k
                    if n == 0:
                        continue
                    # factors F = -H[i,k] * inv(p), via paired products:
                    #   P1 = (a*ir || a*ii),  P2 = (b*ir || b*ii)
                    car = rows3(HRe, k, k + 1, n)
                    cai = rows3(HIm, k, k + 1, n)
                    car4 = car[:, None, :, :].broadcast_to([128, 2, n, M])
                    cai4 = cai[:, None, :, :].broadcast_to([128, 2, n, M])
                    nc.vector.tensor_mul(sc4(PAs, n), car4, inv_pair(k, n))
                    nc.vector.tensor_mul(sc4(PBs, n), cai4, inv_pair(k, n))
                    fre = FRe[:, : n * M]
                    fim = FIm[:, : n * M]
                    # fre = -(a*ir + b*ii), fim = a*ii - b*ir
                    nc.vector.scalar_tensor_tensor(
                        fre, sc_half(PAs, 0, n), -1.0, sc_half(PBs, 1, n),
                        AL.mult, AL.subtract,
                    )
                    nc.vector.tensor_sub(
                        fim, sc_half(PAs, 1, n), sc_half(PBs, 0, n)
                    )
                    # eliminate column k from planes k+1..7 and y, in groups
                    # of up to 4 planes per instruction: the plane index is a
                    # third AP dim (stride A*M), so one [128, w, n, M] op
                    # covers w planes.  Products cycle through the single
                    # scratch PAs; the in-order DVE serializes them anyway.
                    js = list(range(k + 1, NP))
                    while js:
                        w = min(4, len(js))
                        j0 = js[0]
                        js = js[w:]

                        def wrows(T):
                            base = _off(j0, k + 1)
                            return T[:, base : base + w * A * M].rearrange(
                                "p (w c) -> p w c", w=w
                            )[:, :, : n * M]

                        def wrow_b(T):
                            base = _off(j0, k)
                            v = T[:, base : base + w * A * M].rearrange(
                                "p (w c) -> p w c", w=w
                            )[:, :, :M]
                            return v[:, :, None, :].broadcast_to(
                                [128, w, n, M]
                            )

                        def fw(Ft):
                            v = Ft[:, : n * M].rearrange(
                                "p (r c) -> p r c", r=n
                            )
                            return v[:, None, :, :].broadcast_to(
                                [128, w, n, M]
                            )

                        hr, hi = wrows(HRe), wrows(HIm)
                        Br, Bi = wrow_b(HRe), wrow_b(HIm)
                        frew, fimw = fw(FRe), fw(FIm)
                        SA4 = PAs[:, : w * n * M].rearrange(
                            "p (w r c) -> p w r c", w=w, r=n
                        )
                        SA3 = PAs[:, : w * n * M].rearrange(
                            "p (w c) -> p w c", w=w
                        )
                        SB4 = PBs[:, : w * n * M].rearrange(
                            "p (w r c) -> p w r c", w=w, r=n
                        )
                        SB3 = PBs[:, : w * n * M].rearrange(
                            "p (w c) -> p w c", w=w
                        )
                        # H[i,j] += F*B (complex); products regrouped by
                        # factor so consecutive VectorE ops never share a
                        # RAW destination (longer dep gaps -> less ack stall)
                        nc.vector.tensor_mul(SA4, frew, Br)
                        nc.vector.tensor_mul(SB4, frew, Bi)
                        nc.vector.tensor_add(hr, hr, SA3)
                        nc.vector.tensor_add(hi, hi, SB3)
                        nc.vector.tensor_mul(SA4, fimw, Bi)
                        nc.vector.tensor_mul(SB4, fimw, Br)
                        nc.vector.tensor_sub(hr, hr, SA3)
                        nc.vector.tensor_add(hi, hi, SB3)

                # back substitution (Jordan): x_k = y_k*invp, then clear col k
                for k in range(A - 1, -1, -1):
                    yr = row(HRe, 8, k)
                    yi = row(HIm, 8, k)
                    # P1 = (yr*ir || yr*ii), P2 = (yi*ir || yi*ii)
                    p1 = PAs[:, : 2 * M].rearrange("p (j c) -> p j c", j=2)
                    p2 = PBs[:, : 2 * M].rearrange("p (j c) -> p j c", j=2)
                    yr2 = yr[:, None, :].broadcast_to([128, 2, M])
                    yi2 = yi[:, None, :].broadcast_to([128, 2, M])
                    nc.vector.tensor_mul(p1, yr2, inv_pair(k))
                    nc.vector.tensor_mul(p2, yi2, inv_pair(k))
                    # x = y * conj(p)/|p|^2: xr = yr*ir + yi*ii, xi = yi*ir - yr*ii
                    nc.vector.tensor_add(yr, PAs[:, :M], PBs[:, M : 2 * M])
                    nc.vector.tensor_sub(yi, PBs[:, :M], PAs[:, M : 2 * M])
                    # x_k is final now -- store it while the rest of the back
                    # pass still runs on VectorE.
                    so = stgo.tile([M, 2 * FS], F32, tag="so")
                    so3 = so.rearrange("p (f c) -> p f c", c=2)
                    for comp in range(2):
                        po = pso_pool.tile([M, FS], F32, tag="pso")
                        nc.tensor.transpose(
                            po, row(hsup[comp], 8, k), ident[:128, :128]
                        )
                        nc.scalar.copy(so3[:, :, comp], po)
                    dst = out[k, :, b0 : b0 + NB]
                    nc.sync.dma_start(dst, so)
                    if k == 0:
                        continue
                    cr = rows3(HRe, k, 0, k)
                    ci_ = rows3(HIm, k, 0, k)
                    xrB = bc(yr, k)
                    xiB = bc(yi, k)
                    qa, qb, qc, qd = (sc3(t, k) for t in (PAs, PBs, PCs, PDs))
                    nc.vector.tensor_mul(qa, cr, xrB)
                    nc.vector.tensor_mul(qb, ci_, xiB)
                    nc.vector.tensor_mul(qc, cr, xiB)
                    nc.vector.tensor_mul(qd, ci_, xrB)
                    ytr = rows3(HRe, 8, 0, k)
                    yti = rows3(HIm, 8, 0, k)
                    # y_i -= H[i,k] * x_k
                    nc.vector.tensor_sub(ytr, ytr, qa)
                    nc.vector.tensor_add(ytr, ytr, qb)
                    nc.vector.tensor_sub(yti, yti, qc)
                    nc.vector.tensor_sub(yti, yti, qd)


    nc.finalize()
    return nc


_NC_CACHE = None


def _get_nc():
    global _NC_CACHE
    if _NC_CACHE is None:
        _NC_CACHE = _build()
    return _NC_CACHE


def _prep_core(y_re, y_im, h_re, h_im, c):
    """Host-side shard prep for core c: f-slice + block-diagonal extraction."""
    fsl = slice(c * FS, (c + 1) * FS)
    ue = np.arange(U)
    maps = {}
    for name, h in (("hd_re", h_re), ("hd_im", h_im)):
        h6 = h[:, 0, :, :, :, :, fsl].reshape(B, U, A, U, A, S, FS)
        hd = h6[:, ue, :, ue]              # [u, b, i, j, s, f]
        maps[name] = np.ascontiguousarray(
            hd.transpose(2, 0, 1, 4, 3, 5), dtype=np.float32
        )                                   # [i, u, b, s, j, f]
    for name, y in (("yd_re", y_re), ("yd_im", y_im)):
        y5 = y[:, 0, :, :, fsl].reshape(B, U, A, S, FS)   # [b, u, i, s, f]
        maps[name] = np.ascontiguousarray(
            y5.transpose(2, 1, 0, 3, 4), dtype=np.float32
        )                                   # [i, u, b, s, f]
    return maps


def kernel(y_re, y_im, h_re, h_im, **_ignored):
    global LAST_RESULTS
    y_re = np.asarray(y_re, dtype=np.float32)
    y_im = np.asarray(y_im, dtype=np.float32)
    h_re = np.asarray(h_re, dtype=np.float32)
    h_im = np.asarray(h_im, dtype=np.float32)

    nc = _get_nc()
    in_maps = [_prep_core(y_re, y_im, h_re, h_im, c) for c in range(NCORES)]
    trace = bool(int(os.environ.get("BD_TRACE", "0")))
    res = run_bass_kernel_spmd(
        nc, in_maps, core_ids=list(range(NCORES)), trace=trace
    )
    LAST_RESULTS = res
    outs = []
    for r in res.results:
        o = r["out"]                              # [i, u, b, s, f, c]
        o = o.transpose(2, 1, 0, 3, 4, 5)         # [b, u, i, s, f, c]
        outs.append(o.reshape(B, NR, S, FS, 2))
    full = np.concatenate(outs, axis=3)           # [B, NR, S, F, 2]
    return np.ascontiguousarray(full[:, None])    # [B, 1, NR, S, F, 2]



# revision 5
# speedup vs baseline: 1.3104x; 1.3104x over previous
"""Block-diagonal ZF equalizer (nn_BDEqualizer) as a Trainium2 Bass kernel.

Math: for every resource element (b, s, f) and UE u, solve the 8x8 complex
system H_u x_u = y_u where H_u[i, j] = h[b, 0, 8u+i, u, j, s, f] and
y_u[i] = y[b, 0, 8u+i, s, f].  Output x as [B, 1, 32, S, F, 2] (re/im last).

Strategy (data-parallel over the fft axis, per the sharding hint):
  - 8 cores, each owns a contiguous 128-subcarrier slice of F=1024.
  - Host pre-extracts the block-diagonal channel blocks (pure indexing) and
    ships per-core shards hd[B, U, 8, 8, S, 128] / yd[B, U, 8, S, 128].
  - On-chip layout: subcarriers on the 128 SBUF partitions, the other RE
    axes (u, b-pair, s) = 112 along the free dim.  Unpivoted complex
    Gaussian elimination + Jordan back-substitution on 9 augmented planes
    (8 matrix columns + rhs), re/im as separate fp32 tiles.
  - The elimination work (4 products + 4 accumulates per complex MAC, all
    plain tensor_tensor ops thanks to the unnegated factor convention
    F = H[i,k]*conj(p)/|p|^2) is split between the Vector engine and the
    GpSimd (Pool) engine, which run concurrently: a static planner assigns
    whole planes (and boundary-plane row ranges) per elimination step to
    balance DVE (1.04 ns/elem) against Pool (1.98 ns/elem).
  - Software pipelining: at step k DVE first updates pivot plane k+1,
    then immediately computes step k+1's pivot reciprocal (1-cpe
    approximate reciprocal) and factors into double-buffered factor
    tiles, so Pool's step-k+1 work is never factor-starved.
  - TensorE transposes move between the DMA staging layout [(u,b,s), f]
    and the compute layout [f, (u,b,s)]; ScalarE drains PSUM.  Two chunks
    (b in {0,1} then {2,3}) double-buffer load against compute.
"""

import math
import os

import numpy as np

import concourse.bacc as bacc
import concourse.mybir as mybir
from concourse.bass_utils import run_bass_kernel_spmd
from concourse.masks import make_identity
from concourse.tile import TileContext

B, NRX, NR, U, A, S, F = 4, 1, 32, 4, 8, 14, 1024
NCORES = 8
FS = F // NCORES        # 128 subcarriers per core
NB = 2                  # batch entries per chunk
NCH = B // NB           # chunks per core
M = U * NB * S          # 112 RE columns per chunk (u, b, s)
NP = 9                  # augmented planes: 8 matrix columns + rhs
F32 = mybir.dt.float32
AL = mybir.AluOpType

LAST_RESULTS = None     # BassKernelResults of the most recent run (for test.py)

# --- static DVE/Pool work-splitting planner ---------------------------------
U_D = 112 * (1e9 / 0.96e9)          # DVE ns per M-unit (112 elems)
U_P = 112 * (1e9 / 1.2e9) / 0.42    # Pool ns per M-unit
O_D = 60.0                          # DVE per-instruction busy adder
O_P = 95.0                          # Pool q7 launch per instruction
W_D = 3                             # DVE plane-group width
W_P = 2                             # Pool plane-group width


def plan_fwd(k):
    """Pool assignment for elimination step k: (n_full_planes_from_top, rows).

    Pool takes full planes j in (8-npl, 8] plus the top `rs` rows of plane
    8-npl; plane k+1 always stays fully on DVE (it gates step k+1's
    factors).  Chosen to balance modeled engine busy times.
    """
    n = 7 - k
    nplanes = 8 - k
    best, best_t = (0, 0), float("inf")
    for npl in range(0, nplanes):
        j_b = 8 - npl
        max_rs = n - 1 if j_b >= k + 2 else 0
        for rs in range(0, max_rs + 1):
            pool_u = 8 * (n * npl + rs)
            dve_u = (6 + 6 * n) + 8 * n * nplanes - pool_u
            dve_full = nplanes - npl - (1 if rs > 0 else 0)
            dve_i = 10 + 8 * math.ceil(dve_full / W_D) + (8 if rs > 0 else 0)
            pool_i = 8 * math.ceil(npl / W_P) + (8 if rs > 0 else 0)
            t_d = dve_u * U_D + dve_i * O_D
            t_p = pool_u * U_P + pool_i * O_P + (140.0 if pool_u else 0.0)
            t = max(t_d, t_p)
            if t < best_t:
                best_t, best = t, (npl, rs)
    return best


def plan_bwd(k):
    """Pool rows (from the bottom) for the Jordan back pass at step k."""
    best, best_t = 0, float("inf")
    for m in range(0, k):
        t_d = (6 + 8 * (k - m)) * U_D + 14 * O_D
        t_p = 8 * m * U_P + (8 * O_P + 140.0 if m else 0.0)
        t = max(t_d, t_p)
        if t < best_t:
            best_t, best = t, m
    return best


def _off(j, i):
    """Free-dim offset of (plane j, row i) inside an H supertile."""
    return (j * A + i) * M


def _build():
    nc = bacc.Bacc(trn_type="TRN2")

    # Host-prepped layouts, chosen so every per-(chunk, i) DMA slice is
    # stride-collapsible: hd[i, u, b, s, j, f], yd[i, u, b, s, f],
    # out[i, u, b, s, f, c].  (i = matrix row, j = matrix column.)
    hdre = nc.dram_tensor("hd_re", [A, U, B, S, A, FS], F32, kind="ExternalInput")
    hdim = nc.dram_tensor("hd_im", [A, U, B, S, A, FS], F32, kind="ExternalInput")
    ydre = nc.dram_tensor("yd_re", [A, U, B, S, FS], F32, kind="ExternalInput")
    ydim = nc.dram_tensor("yd_im", [A, U, B, S, FS], F32, kind="ExternalInput")
    out = nc.dram_tensor("out", [A, U, B, S, FS, 2], F32, kind="ExternalOutput")

    with TileContext(nc) as tc:
        with (
            tc.tile_pool(name="consts", bufs=1) as consts,
            tc.tile_pool(name="supers", bufs=2) as supers,
            tc.tile_pool(name="work", bufs=1) as work,
            tc.tile_pool(name="stg", bufs=2) as stg,
            tc.tile_pool(name="stgo", bufs=2) as stgo,
            tc.tile_pool(name="psin", bufs=3, space="PSUM") as psin,
            tc.tile_pool(name="psy", bufs=2, space="PSUM") as psy_pool,
            tc.tile_pool(name="pso", bufs=2, space="PSUM") as pso_pool,
        ):
            ident = consts.tile([128, 128], F32)
            make_identity(nc, ident)

            for ci in range(NCH):
                b0 = ci * NB
                HRe = supers.tile([128, (NP + 1) * A * M], F32, tag="HRe")
                HIm = supers.tile([128, (NP + 1) * A * M], F32, tag="HIm")
                hsup = (HRe, HIm)

                def row(T, j, i):
                    return T[:, _off(j, i) : _off(j, i) + M]

                def rows3(T, j, i0, n):
                    base = _off(j, i0)
                    return T[:, base : base + n * M].rearrange(
                        "p (r c) -> p r c", r=n
                    )

                def bc(ap, n):
                    return ap[:, None, :].broadcast_to([128, n, M])

                # ---------------- load h ----------------
                for comp in range(2):
                    hsrc = (hdre, hdim)[comp]
                    for i in range(A):
                        stage = stg.tile([M, A * FS], F32, tag="stage")
                        src = hsrc[i, :, b0 : b0 + NB]
                        nc.sync.dma_start(stage, src)
                        for jg in range(2):
                            ps = psin.tile([128, 4 * M], F32, tag="psin")
                            for q in range(4):
                                j = jg * 4 + q
                                nc.tensor.transpose(
                                    ps[:, q * M : (q + 1) * M],
                                    stage[:, j * FS : (j + 1) * FS],
                                    ident[:M, :M],
                                )
                            base = _off(jg * 4, i)
                            dst = hsup[comp][:, base : base + 4 * A * M].rearrange(
                                "p (q c) -> p q c", q=4
                            )[:, :, :M]
                            src3 = ps.rearrange("p (q c) -> p q c", q=4)
                            nc.scalar.copy(dst, src3)

                # ---------------- load y ----------------
                for comp in range(2):
                    ysrc = (ydre, ydim)[comp]
                    for i in range(A):
                        sy = stg.tile([M, FS], F32, tag="stagey")
                        nc.sync.dma_start(sy, ysrc[i, :, b0 : b0 + NB])
                        py = psy_pool.tile([128, M], F32, tag="psy")
                        nc.tensor.transpose(py, sy, ident[:M, :M])
                        nc.scalar.copy(row(hsup[comp], 8, i), py)

                # ---------------- solve ----------------
                # INV holds the pivot reciprocals: ir block [0:A*M], ii block
                # [A*M:2*A*M], plus A*M padding so the (ir_k, ii_k) stride-
                # A*M pair view can be built by slice+rearrange for every k.
                INV = work.tile([128, 3 * A * M], F32, tag="INV")
                # Double-buffered factor tiles: step k's factors live in
                # F*[k % 2] so step k+1's can be computed early.
                FRe0 = work.tile([128, 7 * M], F32, tag="FRe0")
                FRe1 = work.tile([128, 7 * M], F32, tag="FRe1")
                FIm0 = work.tile([128, 7 * M], F32, tag="FIm0")
                FIm1 = work.tile([128, 7 * M], F32, tag="FIm1")
                FRe = (FRe0, FRe1)
                FIm = (FIm0, FIm1)
                # Per-engine product scratch (the engines run concurrently).
                PAs = work.tile([128, W_D * 7 * M], F32, tag="PAs")
                PBs = work.tile([128, W_D * 7 * M], F32, tag="PBs")
                PPa = work.tile([128, W_P * 7 * M], F32, tag="PPa")
                PPb = work.tile([128, W_P * 7 * M], F32, tag="PPb")
                TD = work.tile([128, M], F32, tag="TD")
                TU = work.tile([128, M], F32, tag="TU")
                TR = work.tile([128, M], F32, tag="TR")

                def inv_pair(k, n=None):
                    # (ir_k, ii_k) as [128, 2, M]; broadcast over n rows if set
                    v = INV[:, k * M : k * M + 2 * A * M].rearrange(
                        "p (j c) -> p j c", j=2
                    )[:, :, :M]
                    if n is None:
                        return v
                    return v[:, :, None, :].broadcast_to([128, 2, n, M])

                def pivot_chain(k):
                    # ir_k + i*ii_k = conj(p)/|p|^2 for pivot p of step k
                    a = row(HRe, k, k)
                    b_ = row(HIm, k, k)
                    nc.vector.tensor_mul(TD, a, a)
                    nc.vector.tensor_mul(TU, b_, b_)
                    nc.vector.tensor_add(TD, TD, TU)
                    nc.vector.reciprocal_approx_fast(TR, TD)
                    irk = INV[:, k * M : (k + 1) * M]
                    iik = INV[:, (A + k) * M : (A + k + 1) * M]
                    nc.vector.tensor_mul(irk, a, TR)
                    nc.vector.tensor_mul(iik, b_, TR)

                def factors(k):
                    # F = H[i,k] * conj(p)/|p|^2 (unnegated) for i in k+1..7
                    n = A - 1 - k
                    car = rows3(HRe, k, k + 1, n)
                    cai = rows3(HIm, k, k + 1, n)
                    car4 = car[:, None, :, :].broadcast_to([128, 2, n, M])
                    cai4 = cai[:, None, :, :].broadcast_to([128, 2, n, M])
                    p1 = PAs[:, : 2 * n * M].rearrange("p (j c) -> p j c", j=2)
                    p2 = PBs[:, : 2 * n * M].rearrange("p (j c) -> p j c", j=2)
                    nc.vector.tensor_mul(p1, car4, inv_pair(k, n))
                    nc.vector.tensor_mul(p2, cai4, inv_pair(k, n))
                    fre = FRe[k % 2][:, : n * M]
                    fim = FIm[k % 2][:, : n * M]
                    # fre = cr*ir + ci*ii, fim = ci*ir - cr*ii
                    nc.vector.tensor_add(
                        fre, PAs[:, : n * M], PBs[:, n * M : 2 * n * M]
                    )
                    nc.vector.tensor_sub(
                        fim, PBs[:, : n * M], PAs[:, n * M : 2 * n * M]
                    )

                def elim_group(eng, sa, sb, k, j0, w, i0, nr):
                    """Eliminate col k from planes [j0, j0+w), rows [i0, i0+nr).

                    H[i,j] -= F_i * H[k,j]:  hr -= fre*Br - fim*Bi,
                                             hi -= fre*Bi + fim*Br.
                    """
                    def wrows(T):
                        base = _off(j0, i0)
                        return T[:, base : base + w * A * M].rearrange(
                            "p (w c) -> p w c", w=w
                        )[:, :, : nr * M]

                    def wrow_b(T):
                        base = _off(j0, k)
                        v = T[:, base : base + w * A * M].rearrange(
                            "p (w c) -> p w c", w=w
                        )[:, :, :M]
                        return v[:, :, None, :].broadcast_to([128, w, nr, M])

                    def fw(Ft):
                        o = (i0 - k - 1) * M
                        v = Ft[:, o : o + nr * M].rearrange(
                            "p (r c) -> p r c", r=nr
                        )
                        return v[:, None, :, :].broadcast_to([128, w, nr, M])

                    hr, hi = wrows(HRe), wrows(HIm)
                    Br, Bi = wrow_b(HRe), wrow_b(HIm)
                    frew, fimw = fw(FRe[k % 2]), fw(FIm[k % 2])
                    sz = w * nr * M
                    A4 = sa[:, :sz].rearrange("p (w r c) -> p w r c", w=w, r=nr)
                    A3 = sa[:, :sz].rearrange("p (w c) -> p w c", w=w)
                    B4 = sb[:, :sz].rearrange("p (w r c) -> p w r c", w=w, r=nr)
                    B3 = sb[:, :sz].rearrange("p (w c) -> p w c", w=w)
                    eng.tensor_mul(A4, frew, Br)
                    eng.tensor_mul(B4, frew, Bi)
                    eng.tensor_sub(hr, hr, A3)
                    eng.tensor_sub(hi, hi, B3)
                    eng.tensor_mul(A4, fimw, Bi)
                    eng.tensor_mul(B4, fimw, Br)
                    eng.tensor_add(hr, hr, A3)
                    eng.tensor_sub(hi, hi, B3)

                def emit_groups(eng, sa, sb, k, planes, i0, nr, wmax):
                    js = list(planes)
                    while js:
                        w = 1
                        while (
                            w < wmax
                            and w < len(js)
                            and js[w] == js[0] + w
                        ):
                            w += 1
                        elim_group(eng, sa, sb, k, js[0], w, i0, nr)
                        js = js[w:]

                # ---------------- forward elimination ----------------
                pivot_chain(0)
                factors(0)
                for k in range(A - 1):
                    n = A - 1 - k
                    npl, rs = plan_fwd(k)
                    j_b = 8 - npl
                    # DVE: pivot plane k+1 first, then next step's prep
                    elim_group(nc.vector, PAs, PBs, k, k + 1, 1, k + 1, n)
                    pivot_chain(k + 1)
                    if k + 1 < A - 1:
                        factors(k + 1)
                    # Pool: full planes from the top + partial rows
                    pool_planes = list(range(j_b + 1, 9))
                    if pool_planes:
                        emit_groups(
                            nc.gpsimd, PPa, PPb, k, pool_planes, k + 1, n, W_P
                        )
                    if rs > 0:
                        elim_group(
                            nc.gpsimd, PPa, PPb, k, j_b, 1, 8 - rs, rs
                        )
                    # DVE: remaining full planes, then the partial remainder
                    dve_full = [j for j in range(k + 2, j_b + (0 if rs else 1))]
                    if dve_full:
                        emit_groups(
                            nc.vector, PAs, PBs, k, dve_full, k + 1, n, W_D
                        )
                    if rs > 0 and (n - rs) > 0:
                        elim_group(
                            nc.vector, PAs, PBs, k, j_b, 1, k + 1, n - rs
                        )

                # ---------------- back substitution (Jordan) ----------------
                for k in range(A - 1, -1, -1):
                    yr = row(HRe, 8, k)
                    yi = row(HIm, 8, k)
                    # x = y * conj(p)/|p|^2
                    p1 = PAs[:, : 2 * M].rearrange("p (j c) -> p j c", j=2)
                    p2 = PBs[:, : 2 * M].rearrange("p (j c) -> p j c", j=2)
                    yr2 = yr[:, None, :].broadcast_to([128, 2, M])
                    yi2 = yi[:, None, :].broadcast_to([128, 2, M])
                    nc.vector.tensor_mul(p1, yr2, inv_pair(k))
                    nc.vector.tensor_mul(p2, yi2, inv_pair(k))
                    # xr = yr*ir + yi*ii, xi = yi*ir - yr*ii
                    nc.vector.tensor_add(yr, PAs[:, :M], PBs[:, M : 2 * M])
                    nc.vector.tensor_sub(yi, PBs[:, :M], PAs[:, M : 2 * M])
                    # x_k is final now -- store it while the rest of the back
                    # pass still runs.
                    so = stgo.tile([M, 2 * FS], F32, tag="so")
                    so3 = so.rearrange("p (f c) -> p f c", c=2)
                    for comp in range(2):
                        po = pso_pool.tile([M, FS], F32, tag="pso")
                        nc.tensor.transpose(
                            po, row(hsup[comp], 8, k), ident[:128, :128]
                        )
                        nc.scalar.copy(so3[:, :, comp], po)
                    dst = out[k, :, b0 : b0 + NB]
                    nc.sync.dma_start(dst, so)
                    if k == 0:
                        continue
                    m = plan_bwd(k)

                    def yupd(eng, sa, sb, r0, nr):
                        # y_i -= H[i,k]*x_k for rows [r0, r0+nr)
                        cr = rows3(HRe, k, r0, nr)
                        ci_ = rows3(HIm, k, r0, nr)
                        xrB = bc(yr, nr)
                        xiB = bc(yi, nr)
                        qa = sa[:, : nr * M].rearrange(
                            "p (r c) -> p r c", r=nr
                        )
                        qc = sa[:, 7 * M : (7 + nr) * M].rearrange(
                            "p (r c) -> p r c", r=nr
                        )
                        qb = sb[:, : nr * M].rearrange(
                            "p (r c) -> p r c", r=nr
                        )
                        qd = sb[:, 7 * M : (7 + nr) * M].rearrange(
                            "p (r c) -> p r c", r=nr
                        )
                        ytr = rows3(HRe, 8, r0, nr)
                        yti = rows3(HIm, 8, r0, nr)
                        eng.tensor_mul(qa, cr, xrB)
                        eng.tensor_mul(qc, cr, xiB)
                        eng.tensor_sub(ytr, ytr, qa)
                        eng.tensor_sub(yti, yti, qc)
                        eng.tensor_mul(qb, ci_, xiB)
                        eng.tensor_mul(qd, ci_, xrB)
                        eng.tensor_add(ytr, ytr, qb)
                        eng.tensor_sub(yti, yti, qd)

                    if m > 0:
                        yupd(nc.gpsimd, PPa, PPb, 0, m)
                    if k - m > 0:
                        yupd(nc.vector, PAs, PBs, m, k - m)

    nc.finalize()
    return nc


_NC_CACHE = None


def _get_nc():
    global _NC_CACHE
    if _NC_CACHE is None:
        _NC_CACHE = _build()
    return _NC_CACHE


def _prep_core(y_re, y_im, h_re, h_im, c):
    """Host-side shard prep for core c: f-slice + block-diagonal extraction."""
    fsl = slice(c * FS, (c + 1) * FS)
    ue = np.arange(U)
    maps = {}
    for name, h in (("hd_re", h_re), ("hd_im", h_im)):
        h6 = h[:, 0, :, :, :, :, fsl].reshape(B, U, A, U, A, S, FS)
        hd = h6[:, ue, :, ue]              # [u, b, i, j, s, f]
        maps[name] = np.ascontiguousarray(
            hd.transpose(2, 0, 1, 4, 3, 5), dtype=np.float32
        )                                   # [i, u, b, s, j, f]
    for name, y in (("yd_re", y_re), ("yd_im", y_im)):
        y5 = y[:, 0, :, :, fsl].reshape(B, U, A, S, FS)   # [b, u, i, s, f]
        maps[name] = np.ascontiguousarray(
            y5.transpose(2, 1, 0, 3, 4), dtype=np.float32
        )                                   # [i, u, b, s, f]
    return maps


def kernel(y_re, y_im, h_re, h_im, **_ignored):
    global LAST_RESULTS
    y_re = np.asarray(y_re, dtype=np.float32)
    y_im = np.asarray(y_im, dtype=np.float32)
    h_re = np.asarray(h_re, dtype=np.float32)
    h_im = np.asarray(h_im, dtype=np.float32)

    nc = _get_nc()
    in_maps = [_prep_core(y_re, y_im, h_re, h_im, c) for c in range(NCORES)]
    trace = bool(int(os.environ.get("BD_TRACE", "0")))
    res = run_bass_kernel_spmd(
        nc, in_maps, core_ids=list(range(NCORES)), trace=trace
    )
    LAST_RESULTS = res
    outs = []
    for r in res.results:
        o = r["out"]                              # [i, u, b, s, f, c]
        o = o.transpose(2, 1, 0, 3, 4, 5)         # [b, u, i, s, f, c]
        outs.append(o.reshape(B, NR, S, FS, 2))
    full = np.concatenate(outs, axis=3)           # [B, NR, S, F, 2]
    return np.ascontiguousarray(full[:, None])    # [B, 1, NR, S, F, 2]


# revision 9
# speedup vs baseline: 1.3790x; 1.0523x over previous
"""Block-diagonal ZF equalizer (nn_BDEqualizer) as a Trainium2 Bass kernel.

Math: for every resource element (b, s, f) and UE u, solve the 8x8 complex
system H_u x_u = y_u where H_u[i, j] = h[b, 0, 8u+i, u, j, s, f] and
y_u[i] = y[b, 0, 8u+i, s, f].  Output x as [B, 1, 32, S, F, 2] (re/im last).

Strategy (data-parallel over the fft axis, per the sharding hint):
  - 8 cores, each owns a contiguous 128-subcarrier slice of F=1024.
  - Host pre-extracts the block-diagonal channel blocks (pure indexing) and
    ships per-core shards hd[B, U, 8, 8, S, 128] / yd[B, U, 8, S, 128].
  - On-chip layout: subcarriers on the 128 SBUF partitions, the other RE
    axes (u, b-pair, s) = 112 along the free dim.  Unpivoted complex
    Gaussian elimination + Jordan back-substitution on 9 augmented planes
    (8 matrix columns + rhs), re/im as separate fp32 tiles.
  - The elimination work (4 products + 4 accumulates per complex MAC, all
    plain tensor_tensor ops thanks to the unnegated factor convention
    F = H[i,k]*conj(p)/|p|^2) is split between the Vector engine and the
    GpSimd (Pool) engine, which run concurrently: a static planner assigns
    whole planes (and boundary-plane row ranges) per elimination step to
    balance DVE (1.04 ns/elem) against Pool (1.98 ns/elem).
  - Software pipelining: at step k DVE first updates pivot plane k+1,
    then immediately computes step k+1's pivot reciprocal (1-cpe
    approximate reciprocal) and factors into double-buffered factor
    tiles, so Pool's step-k+1 work is never factor-starved.
  - TensorE transposes move between the DMA staging layout [(u,b,s), f]
    and the compute layout [f, (u,b,s)]; ScalarE drains PSUM.  Two chunks
    (b in {0,1} then {2,3}) double-buffer load against compute.
"""

import math
import os

import numpy as np

import concourse.bacc as bacc
import concourse.mybir as mybir
from concourse.bass_utils import run_bass_kernel_spmd
from concourse.masks import make_identity
from concourse.tile import TileContext

B, NRX, NR, U, A, S, F = 4, 1, 32, 4, 8, 14, 1024
NCORES = 8
FS = F // NCORES        # 128 subcarriers per core
NB = 2                  # batch entries per chunk
NCH = B // NB           # chunks per core
M = U * NB * S          # 112 RE columns per chunk (u, b, s)
NP = 9                  # augmented planes: 8 matrix columns + rhs
F32 = mybir.dt.float32
AL = mybir.AluOpType

LAST_RESULTS = None     # BassKernelResults of the most recent run (for test.py)

# --- static DVE/Pool work-splitting planner ---------------------------------
U_D = 112 * (1e9 / 0.96e9)          # DVE ns per M-unit (112 elems)
U_P = 112 * (1e9 / 1.2e9) / 0.42    # Pool ns per M-unit
O_D = 60.0                          # DVE per-instruction busy adder
O_P = 95.0                          # Pool q7 launch per instruction
W_D = 3                             # DVE plane-group width
W_P = 2                             # Pool plane-group width


def plan_fwd(k):
    """Pool assignment for elimination step k: (n_full_planes_from_top, rows).

    Pool takes full planes j in (8-npl, 8] plus the top `rs` rows of plane
    8-npl; plane k+1 always stays fully on DVE (it gates step k+1's
    factors).  Chosen to balance modeled engine busy times.
    """
    n = 7 - k
    nplanes = 8 - k
    best, best_t = (0, 0), float("inf")
    for npl in range(0, nplanes):
        j_b = 8 - npl
        max_rs = n - 1 if j_b >= k + 2 else 0
        for rs in range(0, max_rs + 1):
            pool_u = 8 * (n * npl + rs)
            dve_u = (6 + 6 * n) + 8 * n * nplanes - pool_u
            dve_full = nplanes - npl - (1 if rs > 0 else 0)
            dve_i = 10 + 8 * math.ceil(dve_full / W_D) + (8 if rs > 0 else 0)
            pool_i = 8 * math.ceil(npl / W_P) + (8 if rs > 0 else 0)
            t_d = dve_u * U_D + dve_i * O_D
            t_p = pool_u * U_P + pool_i * O_P + (140.0 if pool_u else 0.0)
            t = max(t_d, t_p)
            if t < best_t:
                best_t, best = t, (npl, rs)
    return best


def plan_bwd(k):
    """Pool rows (from the bottom) for the Jordan back pass at step k."""
    best, best_t = 0, float("inf")
    for m in range(0, k):
        t_d = (6 + 8 * (k - m)) * U_D + 14 * O_D
        t_p = 8 * m * U_P + (8 * O_P + 140.0 if m else 0.0)
        t = max(t_d, t_p)
        if t < best_t:
            best_t, best = t, m
    return best


def _off(j, i):
    """Free-dim offset of (plane j, row i) inside an H supertile."""
    return (j * A + i) * M


def _build():
    nc = bacc.Bacc(trn_type="TRN2")

    # Host-prepped layouts, plane-major so one DMA delivers one full matrix
    # column-plane and the solve can start after ~2 plane loads:
    # hd[j, u, b, s, i, f], yd[u, b, s, i, f], out[i, u, b, s, f, c].
    # (i = matrix row, j = matrix column.)
    hdre = nc.dram_tensor("hd_re", [A, U, B, S, A, FS], F32, kind="ExternalInput")
    hdim = nc.dram_tensor("hd_im", [A, U, B, S, A, FS], F32, kind="ExternalInput")
    ydre = nc.dram_tensor("yd_re", [U, B, S, A, FS], F32, kind="ExternalInput")
    ydim = nc.dram_tensor("yd_im", [U, B, S, A, FS], F32, kind="ExternalInput")
    out = nc.dram_tensor("out", [A, U, B, S, FS, 2], F32, kind="ExternalOutput")

    with TileContext(nc) as tc:
        with (
            tc.tile_pool(name="consts", bufs=1) as consts,
            tc.tile_pool(name="supers", bufs=2) as supers,
            tc.tile_pool(name="work", bufs=1) as work,
            tc.tile_pool(name="stg", bufs=2) as stg,
            tc.tile_pool(name="stgo", bufs=2) as stgo,
            tc.tile_pool(name="psin", bufs=3, space="PSUM") as psin,
            tc.tile_pool(name="pso", bufs=2, space="PSUM") as pso_pool,
        ):
            ident = consts.tile([128, 128], F32)
            make_identity(nc, ident)

            for ci in range(NCH):
                b0 = ci * NB
                HRe = supers.tile([128, (NP + 1) * A * M], F32, tag="HRe")
                HIm = supers.tile([128, (NP + 1) * A * M], F32, tag="HIm")
                hsup = (HRe, HIm)

                def row(T, j, i):
                    return T[:, _off(j, i) : _off(j, i) + M]

                def rows3(T, j, i0, n):
                    base = _off(j, i0)
                    return T[:, base : base + n * M].rearrange(
                        "p (r c) -> p r c", r=n
                    )

                def bc(ap, n):
                    return ap[:, None, :].broadcast_to([128, n, M])

                # ---------------- load h and y, plane-major ----------------
                # Plane order matches consumption: DVE needs 0,1,2 first,
                # Pool's first groups touch 6,7,8(y), the rest follow.
                for j in (0, 1, 2, 6, 7, 8, 3, 4, 5):
                    for comp in range(2):
                        if j == 8:
                            src = (ydre, ydim)[comp][:, b0 : b0 + NB]
                        else:
                            src = (hdre, hdim)[comp][j, :, b0 : b0 + NB]
                        stage = stg.tile([M, A * FS], F32, tag="stage")
                        nc.sync.dma_start(stage, src)
                        for ig in range(2):
                            ps = psin.tile([128, 4 * M], F32, tag="psin")
                            for q in range(4):
                                i = ig * 4 + q
                                nc.tensor.transpose(
                                    ps[:, q * M : (q + 1) * M],
                                    stage[:, i * FS : (i + 1) * FS],
                                    ident[:M, :M],
                                )
                            base = _off(j, ig * 4)
                            nc.scalar.copy(
                                hsup[comp][:, base : base + 4 * M], ps
                            )

                # ---------------- solve ----------------
                # INV holds the pivot reciprocals: ir block [0:A*M], ii block
                # [A*M:2*A*M], plus A*M padding so the (ir_k, ii_k) stride-
                # A*M pair view can be built by slice+rearrange for every k.
                INV = work.tile([128, 3 * A * M], F32, tag="INV")
                # Double-buffered factor tiles: step k's factors live in
                # F*[k % 2] so step k+1's can be computed early.
                FRe0 = work.tile([128, 7 * M], F32, tag="FRe0")
                FRe1 = work.tile([128, 7 * M], F32, tag="FRe1")
                FIm0 = work.tile([128, 7 * M], F32, tag="FIm0")
                FIm1 = work.tile([128, 7 * M], F32, tag="FIm1")
                FRe = (FRe0, FRe1)
                FIm = (FIm0, FIm1)
                # Per-engine product scratch (the engines run concurrently).
                PAs = work.tile([128, W_D * 7 * M], F32, tag="PAs")
                PBs = work.tile([128, W_D * 7 * M], F32, tag="PBs")
                PPa = work.tile([128, W_P * 7 * M], F32, tag="PPa")
                PPb = work.tile([128, W_P * 7 * M], F32, tag="PPb")
                TD = work.tile([128, M], F32, tag="TD")
                TU = work.tile([128, M], F32, tag="TU")
                TR = work.tile([128, M], F32, tag="TR")

                def inv_pair(k, n=None):
                    # (ir_k, ii_k) as [128, 2, M]; broadcast over n rows if set
                    v = INV[:, k * M : k * M + 2 * A * M].rearrange(
                        "p (j c) -> p j c", j=2
                    )[:, :, :M]
                    if n is None:
                        return v
                    return v[:, :, None, :].broadcast_to([128, 2, n, M])

                def pivot_chain(k):
                    # ir_k + i*ii_k = conj(p)/|p|^2 for pivot p of step k
                    a = row(HRe, k, k)
                    b_ = row(HIm, k, k)
                    nc.vector.tensor_mul(TD, a, a)
                    nc.vector.tensor_mul(TU, b_, b_)
                    nc.vector.tensor_add(TD, TD, TU)
                    nc.vector.reciprocal_approx_fast(TR, TD)
                    irk = INV[:, k * M : (k + 1) * M]
                    iik = INV[:, (A + k) * M : (A + k + 1) * M]
                    nc.vector.tensor_mul(irk, a, TR)
                    nc.vector.tensor_mul(iik, b_, TR)

                def factors(k):
                    # F = H[i,k] * conj(p)/|p|^2 (unnegated) for i in k+1..7
                    n = A - 1 - k
                    car = rows3(HRe, k, k + 1, n)
                    cai = rows3(HIm, k, k + 1, n)
                    car4 = car[:, None, :, :].broadcast_to([128, 2, n, M])
                    cai4 = cai[:, None, :, :].broadcast_to([128, 2, n, M])
                    p1 = PAs[:, : 2 * n * M].rearrange("p (j c) -> p j c", j=2)
                    p2 = PBs[:, : 2 * n * M].rearrange("p (j c) -> p j c", j=2)
                    nc.vector.tensor_mul(p1, car4, inv_pair(k, n))
                    nc.vector.tensor_mul(p2, cai4, inv_pair(k, n))
                    fre = FRe[k % 2][:, : n * M]
                    fim = FIm[k % 2][:, : n * M]
                    # fre = cr*ir + ci*ii, fim = ci*ir - cr*ii
                    nc.vector.tensor_add(
                        fre, PAs[:, : n * M], PBs[:, n * M : 2 * n * M]
                    )
                    nc.vector.tensor_sub(
                        fim, PBs[:, : n * M], PAs[:, n * M : 2 * n * M]
                    )

                def elim_group(eng, sa, sb, k, j0, w, i0, nr):
                    """Eliminate col k from planes [j0, j0+w), rows [i0, i0+nr).

                    H[i,j] -= F_i * H[k,j]:  hr -= fre*Br - fim*Bi,
                                             hi -= fre*Bi + fim*Br.
                    """
                    def wrows(T):
                        base = _off(j0, i0)
                        return T[:, base : base + w * A * M].rearrange(
                            "p (w c) -> p w c", w=w
                        )[:, :, : nr * M]

                    def wrow_b(T):
                        base = _off(j0, k)
                        v = T[:, base : base + w * A * M].rearrange(
                            "p (w c) -> p w c", w=w
                        )[:, :, :M]
                        return v[:, :, None, :].broadcast_to([128, w, nr, M])

                    def fw(Ft):
                        o = (i0 - k - 1) * M
                        v = Ft[:, o : o + nr * M].rearrange(
                            "p (r c) -> p r c", r=nr
                        )
                        return v[:, None, :, :].broadcast_to([128, w, nr, M])

                    hr, hi = wrows(HRe), wrows(HIm)
                    Br, Bi = wrow_b(HRe), wrow_b(HIm)
                    frew, fimw = fw(FRe[k % 2]), fw(FIm[k % 2])
                    sz = w * nr * M
                    A4 = sa[:, :sz].rearrange("p (w r c) -> p w r c", w=w, r=nr)
                    A3 = sa[:, :sz].rearrange("p (w c) -> p w c", w=w)
                    B4 = sb[:, :sz].rearrange("p (w r c) -> p w r c", w=w, r=nr)
                    B3 = sb[:, :sz].rearrange("p (w c) -> p w c", w=w)
                    eng.tensor_mul(A4, frew, Br)
                    eng.tensor_mul(B4, frew, Bi)
                    eng.tensor_sub(hr, hr, A3)
                    eng.tensor_sub(hi, hi, B3)
                    eng.tensor_mul(A4, fimw, Bi)
                    eng.tensor_mul(B4, fimw, Br)
                    eng.tensor_add(hr, hr, A3)
                    eng.tensor_sub(hi, hi, B3)

                def emit_groups(eng, sa, sb, k, planes, i0, nr, wmax):
                    js = list(planes)
                    while js:
                        w = 1
                        while (
                            w < wmax
                            and w < len(js)
                            and js[w] == js[0] + w
                        ):
                            w += 1
                        elim_group(eng, sa, sb, k, js[0], w, i0, nr)
                        js = js[w:]

                # ---------------- forward elimination ----------------
                pivot_chain(0)
                factors(0)
                for k in range(A - 1):
                    n = A - 1 - k
                    npl, rs = plan_fwd(k)
                    j_b = 8 - npl
                    # DVE: pivot plane k+1 first, then next step's prep
                    elim_group(nc.vector, PAs, PBs, k, k + 1, 1, k + 1, n)
                    pivot_chain(k + 1)
                    if k + 1 < A - 1:
                        factors(k + 1)
                    # Pool: full planes from the top + partial rows
                    pool_planes = list(range(j_b + 1, 9))
                    if pool_planes:
                        emit_groups(
                            nc.gpsimd, PPa, PPb, k, pool_planes, k + 1, n, W_P
                        )
                    if rs > 0:
                        elim_group(
                            nc.gpsimd, PPa, PPb, k, j_b, 1, 8 - rs, rs
                        )
                    # DVE: remaining full planes, then the partial remainder
                    dve_full = [j for j in range(k + 2, j_b + (0 if rs else 1))]
                    if dve_full:
                        emit_groups(
                            nc.vector, PAs, PBs, k, dve_full, k + 1, n, W_D
                        )
                    if rs > 0 and (n - rs) > 0:
                        elim_group(
                            nc.vector, PAs, PBs, k, j_b, 1, k + 1, n - rs
                        )

                # ---------------- back substitution (Jordan) ----------------
                for k in range(A - 1, -1, -1):
                    yr = row(HRe, 8, k)
                    yi = row(HIm, 8, k)
                    # x = y * conj(p)/|p|^2
                    p1 = PAs[:, : 2 * M].rearrange("p (j c) -> p j c", j=2)
                    p2 = PBs[:, : 2 * M].rearrange("p (j c) -> p j c", j=2)
                    yr2 = yr[:, None, :].broadcast_to([128, 2, M])
                    yi2 = yi[:, None, :].broadcast_to([128, 2, M])
                    nc.vector.tensor_mul(p1, yr2, inv_pair(k))
                    nc.vector.tensor_mul(p2, yi2, inv_pair(k))
                    # xr = yr*ir + yi*ii, xi = yi*ir - yr*ii
                    nc.vector.tensor_add(yr, PAs[:, :M], PBs[:, M : 2 * M])
                    nc.vector.tensor_sub(yi, PBs[:, :M], PAs[:, M : 2 * M])
                    # x_k is final now -- store it while the rest of the back
                    # pass still runs.
                    so = stgo.tile([M, 2 * FS], F32, tag="so")
                    so3 = so.rearrange("p (f c) -> p f c", c=2)
                    for comp in range(2):
                        po = pso_pool.tile([M, FS], F32, tag="pso")
                        nc.tensor.transpose(
                            po, row(hsup[comp], 8, k), ident[:128, :128]
                        )
                        nc.scalar.copy(so3[:, :, comp], po)
                    dst = out[k, :, b0 : b0 + NB]
                    nc.sync.dma_start(dst, so)
                    if k == 0:
                        continue
                    m = plan_bwd(k)

                    def yupd(eng, sa, sb, r0, nr):
                        # y_i -= H[i,k]*x_k for rows [r0, r0+nr)
                        cr = rows3(HRe, k, r0, nr)
                        ci_ = rows3(HIm, k, r0, nr)
                        xrB = bc(yr, nr)
                        xiB = bc(yi, nr)
                        qa = sa[:, : nr * M].rearrange(
                            "p (r c) -> p r c", r=nr
                        )
                        qc = sa[:, 7 * M : (7 + nr) * M].rearrange(
                            "p (r c) -> p r c", r=nr
                        )
                        qb = sb[:, : nr * M].rearrange(
                            "p (r c) -> p r c", r=nr
                        )
                        qd = sb[:, 7 * M : (7 + nr) * M].rearrange(
                            "p (r c) -> p r c", r=nr
                        )
                        ytr = rows3(HRe, 8, r0, nr)
                        yti = rows3(HIm, 8, r0, nr)
                        eng.tensor_mul(qa, cr, xrB)
                        eng.tensor_mul(qc, cr, xiB)
                        eng.tensor_sub(ytr, ytr, qa)
                        eng.tensor_sub(yti, yti, qc)
                        eng.tensor_mul(qb, ci_, xiB)
                        eng.tensor_mul(qd, ci_, xrB)
                        eng.tensor_add(ytr, ytr, qb)
                        eng.tensor_sub(yti, yti, qd)

                    if m > 0:
                        yupd(nc.gpsimd, PPa, PPb, 0, m)
                    if k - m > 0:
                        yupd(nc.vector, PAs, PBs, m, k - m)

    nc.finalize()
    return nc


_NC_CACHE = None


def _get_nc():
    global _NC_CACHE
    if _NC_CACHE is None:
        _NC_CACHE = _build()
    return _NC_CACHE


def _prep_core(y_re, y_im, h_re, h_im, c):
    """Host-side shard prep for core c: f-slice + block-diagonal extraction."""
    fsl = slice(c * FS, (c + 1) * FS)
    ue = np.arange(U)
    maps = {}
    for name, h in (("hd_re", h_re), ("hd_im", h_im)):
        h6 = h[:, 0, :, :, :, :, fsl].reshape(B, U, A, U, A, S, FS)
        hd = h6[:, ue, :, ue]              # [u, b, i, j, s, f]
        maps[name] = np.ascontiguousarray(
            hd.transpose(3, 0, 1, 4, 2, 5), dtype=np.float32
        )                                   # [j, u, b, s, i, f]
    for name, y in (("yd_re", y_re), ("yd_im", y_im)):
        y5 = y[:, 0, :, :, fsl].reshape(B, U, A, S, FS)   # [b, u, i, s, f]
        maps[name] = np.ascontiguousarray(
            y5.transpose(1, 0, 3, 2, 4), dtype=np.float32
        )                                   # [u, b, s, i, f]
    return maps


def kernel(y_re, y_im, h_re, h_im, **_ignored):
    global LAST_RESULTS
    y_re = np.asarray(y_re, dtype=np.float32)
    y_im = np.asarray(y_im, dtype=np.float32)
    h_re = np.asarray(h_re, dtype=np.float32)
    h_im = np.asarray(h_im, dtype=np.float32)

    nc = _get_nc()
    in_maps = [_prep_core(y_re, y_im, h_re, h_im, c) for c in range(NCORES)]
    trace = bool(int(os.environ.get("BD_TRACE", "0")))
    res = run_bass_kernel_spmd(
        nc, in_maps, core_ids=list(range(NCORES)), trace=trace
    )
    LAST_RESULTS = res
    outs = []
    for r in res.results:
        o = r["out"]                              # [i, u, b, s, f, c]
        o = o.transpose(2, 1, 0, 3, 4, 5)         # [b, u, i, s, f, c]
        outs.append(o.reshape(B, NR, S, FS, 2))
    full = np.concatenate(outs, axis=3)           # [B, NR, S, F, 2]
    return np.ascontiguousarray(full[:, None])    # [B, 1, NR, S, F, 2]


# revision 11
# speedup vs baseline: 1.4052x; 1.0190x over previous
"""Block-diagonal ZF equalizer (nn_BDEqualizer) as a Trainium2 Bass kernel.

Math: for every resource element (b, s, f) and UE u, solve the 8x8 complex
system H_u x_u = y_u where H_u[i, j] = h[b, 0, 8u+i, u, j, s, f] and
y_u[i] = y[b, 0, 8u+i, s, f].  Output x as [B, 1, 32, S, F, 2] (re/im last).

Strategy (data-parallel over the fft axis, per the sharding hint):
  - 8 cores, each owns a contiguous 128-subcarrier slice of F=1024.
  - Host pre-extracts the block-diagonal channel blocks (pure indexing) and
    ships per-core shards, plane-major so one DMA delivers one full matrix
    column-plane and the solve starts after ~2 plane loads.
  - On-chip layout: subcarriers on the 128 SBUF partitions, the other RE
    axes (u, b-pair, s) = 112 along the free dim.  Unpivoted complex
    Gaussian elimination + Jordan back-substitution on 9 augmented planes
    (8 matrix columns + rhs), re/im as separate fp32 tiles.
  - The elimination work (4 products + 4 accumulates per complex MAC, all
    plain tensor_tensor ops thanks to the unnegated factor convention
    F = H[i,k]*conj(p)/|p|^2) is split between the Vector engine and the
    GpSimd (Pool) engine, which run concurrently: a static planner assigns
    whole planes (and boundary-plane row ranges) per elimination step to
    balance DVE (1.04 ns/elem) against Pool (1.98 ns/elem).
  - Software pipelining within a chunk: at step k DVE first updates pivot
    plane k+1, then immediately computes step k+1's pivot reciprocal
    (1-cpe approximate reciprocal) and factors into double-buffered factor
    tiles, so Pool's step-k+1 work is never factor-starved.
  - Software pipelining across chunks: chunk 0's back-substitution runs
    DVE-only, its steps interleaved on the DVE queue with chunk 1's
    forward steps, while Pool absorbs a biased (larger) share of chunk
    1's early forward work.  This hides both the serial x-chain of the
    back pass and Pool's idle time there.
"""

import math
import os

import numpy as np

import concourse.bacc as bacc
import concourse.mybir as mybir
from concourse.bass_utils import run_bass_kernel_spmd
from concourse.masks import make_identity
from concourse.tile import TileContext

B, NRX, NR, U, A, S, F = 4, 1, 32, 4, 8, 14, 1024
NCORES = 8
FS = F // NCORES        # 128 subcarriers per core
NB = 2                  # batch entries per chunk
NCH = B // NB           # chunks per core
M = U * NB * S          # 112 RE columns per chunk (u, b, s)
NP = 9                  # augmented planes: 8 matrix columns + rhs
F32 = mybir.dt.float32
AL = mybir.AluOpType

LAST_RESULTS = None     # BassKernelResults of the most recent run (for test.py)

# --- static DVE/Pool work-splitting planner ---------------------------------
U_D = 112 * (1e9 / 0.96e9)          # DVE ns per M-unit (112 elems)
U_P = 112 * (1e9 / 1.2e9) / 0.42    # Pool ns per M-unit
O_D = 60.0                          # DVE per-instruction busy adder
O_P = 95.0                          # Pool q7 launch per instruction
W_D = 2                             # DVE plane-group width
W_P = 2                             # Pool plane-group width


def bwd_step_cost(j):
    """Modeled DVE time of a DVE-only Jordan back step with j rows."""
    return (6 + 8 * j) * U_D + (4 + 8 * (1 if j else 0)) * O_D


def plan_fwd(k, bias=0.0):
    """Pool assignment for elimination step k: (n_full_planes_from_top, rows).

    Pool takes full planes j in (8-npl, 8] plus the top `rs` rows of plane
    8-npl; plane k+1 always stays fully on DVE (it gates step k+1's
    factors).  Chosen to balance modeled engine busy times; `bias` is
    extra modeled DVE time (e.g. an interleaved back-sub step of the
    previous chunk) that Pool should absorb.
    """
    n = 7 - k
    nplanes = 8 - k
    best, best_t = (0, 0), float("inf")
    for npl in range(0, nplanes):
        j_b = 8 - npl
        max_rs = n - 1 if j_b >= k + 2 else 0
        for rs in range(0, max_rs + 1):
            pool_u = 8 * (n * npl + rs)
            dve_u = (6 + 6 * n) + 8 * n * nplanes - pool_u
            dve_full = nplanes - npl - (1 if rs > 0 else 0)
            dve_i = 10 + 8 * math.ceil(dve_full / W_D) + (8 if rs > 0 else 0)
            pool_i = 8 * math.ceil(npl / W_P) + (8 if rs > 0 else 0)
            t_d = dve_u * U_D + dve_i * O_D + bias
            t_p = pool_u * U_P + pool_i * O_P + (140.0 if pool_u else 0.0)
            t = max(t_d, t_p)
            if t < best_t:
                best_t, best = t, (npl, rs)
    return best


def plan_bwd(k):
    """Pool rows (from the bottom) for the Jordan back pass at step k."""
    best, best_t = 0, float("inf")
    for m in range(0, k):
        t_d = (6 + 8 * (k - m)) * U_D + 14 * O_D
        t_p = 8 * m * U_P + (8 * O_P + 140.0 if m else 0.0)
        t = max(t_d, t_p)
        if t < best_t:
            best_t, best = t, m
    return best


def _off(j, i):
    """Free-dim offset of (plane j, row i) inside an H supertile."""
    return (j * A + i) * M


def _build():
    nc = bacc.Bacc(trn_type="TRN2")

    # Host-prepped layouts, plane-major: hd[j, u, b, s, i, f],
    # yd[u, b, s, i, f], out[i, u, b, s, f, c].  (i = row, j = column.)
    hdre = nc.dram_tensor("hd_re", [A, U, B, S, A, FS], F32, kind="ExternalInput")
    hdim = nc.dram_tensor("hd_im", [A, U, B, S, A, FS], F32, kind="ExternalInput")
    ydre = nc.dram_tensor("yd_re", [U, B, S, A, FS], F32, kind="ExternalInput")
    ydim = nc.dram_tensor("yd_im", [U, B, S, A, FS], F32, kind="ExternalInput")
    out = nc.dram_tensor("out", [A, U, B, S, FS, 2], F32, kind="ExternalOutput")

    with TileContext(nc) as tc:
        with (
            tc.tile_pool(name="consts", bufs=1) as consts,
            tc.tile_pool(name="supers", bufs=2) as supers,
            tc.tile_pool(name="work", bufs=1) as work,
            tc.tile_pool(name="stg", bufs=2) as stg,
            tc.tile_pool(name="stgo", bufs=2) as stgo,
            tc.tile_pool(name="psin", bufs=3, space="PSUM") as psin,
            tc.tile_pool(name="pso", bufs=2, space="PSUM") as pso_pool,
        ):
            ident = consts.tile([128, 128], F32)
            make_identity(nc, ident)

            # Shared work tiles (single-buffered; engines are in-order so
            # same-engine reuse is safe, and the two engines use disjoint
            # scratch).  INV holds pivot reciprocals per chunk parity at
            # base 0 / 2*A*M (qr block | qi block, + A*M view padding).
            INV = work.tile([128, 5 * A * M], F32, tag="INV")
            FRe0 = work.tile([128, 7 * M], F32, tag="FRe0")
            FRe1 = work.tile([128, 7 * M], F32, tag="FRe1")
            FIm0 = work.tile([128, 7 * M], F32, tag="FIm0")
            FIm1 = work.tile([128, 7 * M], F32, tag="FIm1")
            FRe = (FRe0, FRe1)
            FIm = (FIm0, FIm1)
            PAs = work.tile([128, W_D * 7 * M], F32, tag="PAs")
            PBs = work.tile([128, W_D * 7 * M], F32, tag="PBs")
            PPa = work.tile([128, W_P * 7 * M], F32, tag="PPa")
            PPb = work.tile([128, W_P * 7 * M], F32, tag="PPb")
            TD = work.tile([128, M], F32, tag="TD")
            TU = work.tile([128, M], F32, tag="TU")
            TR = work.tile([128, M], F32, tag="TR")

            def make_chunk(ci):
                HRe = supers.tile([128, (NP + 1) * A * M], F32, tag="HRe")
                HIm = supers.tile([128, (NP + 1) * A * M], F32, tag="HIm")
                return {
                    "ci": ci,
                    "b0": ci * NB,
                    "HRe": HRe,
                    "HIm": HIm,
                    "ibase": (ci % 2) * 2 * A * M,
                }

            def row(T, j, i):
                return T[:, _off(j, i) : _off(j, i) + M]

            def rows3(T, j, i0, n):
                base = _off(j, i0)
                return T[:, base : base + n * M].rearrange("p (r c) -> p r c", r=n)

            def bc(ap, n):
                return ap[:, None, :].broadcast_to([128, n, M])

            def emit_load(C):
                # Plane order matches consumption: DVE needs 0,1,2 first,
                # Pool's first groups touch 6,7 then 8(y).
                b0 = C["b0"]
                for j in (0, 1, 6, 7, 2, 8, 3, 4, 5):
                    for comp in range(2):
                        if j == 8:
                            src = (ydre, ydim)[comp][:, b0 : b0 + NB]
                        else:
                            src = (hdre, hdim)[comp][j, :, b0 : b0 + NB]
                        stage = stg.tile([M, A * FS], F32, tag="stage")
                        nc.sync.dma_start(stage, src)
                        for ig in range(2):
                            ps = psin.tile([128, 4 * M], F32, tag="psin")
                            for q in range(4):
                                i = ig * 4 + q
                                nc.tensor.transpose(
                                    ps[:, q * M : (q + 1) * M],
                                    stage[:, i * FS : (i + 1) * FS],
                                    ident[:M, :M],
                                )
                            base = _off(j, ig * 4)
                            nc.scalar.copy(
                                C[("HRe", "HIm")[comp]][:, base : base + 4 * M],
                                ps,
                            )

            def inv_pair(C, k, n=None):
                # (ir_k, ii_k) as [128, 2, M]; broadcast over n rows if set
                b = C["ibase"]
                v = INV[:, b + k * M : b + k * M + 2 * A * M].rearrange(
                    "p (j c) -> p j c", j=2
                )[:, :, :M]
                if n is None:
                    return v
                return v[:, :, None, :].broadcast_to([128, 2, n, M])

            def pivot_chain(C, k):
                # ir_k + i*ii_k = conj(p)/|p|^2 for pivot p of step k
                b = C["ibase"]
                a = row(C["HRe"], k, k)
                b_ = row(C["HIm"], k, k)
                nc.vector.tensor_mul(TD, a, a)
                nc.vector.tensor_mul(TU, b_, b_)
                nc.vector.tensor_add(TD, TD, TU)
                nc.vector.reciprocal_approx_fast(TR, TD)
                irk = INV[:, b + k * M : b + (k + 1) * M]
                iik = INV[:, b + (A + k) * M : b + (A + k + 1) * M]
                nc.vector.tensor_mul(irk, a, TR)
                nc.vector.tensor_mul(iik, b_, TR)

            def factors(C, k):
                # F = H[i,k] * conj(p)/|p|^2 (unnegated) for i in k+1..7
                n = A - 1 - k
                car = rows3(C["HRe"], k, k + 1, n)
                cai = rows3(C["HIm"], k, k + 1, n)
                car4 = car[:, None, :, :].broadcast_to([128, 2, n, M])
                cai4 = cai[:, None, :, :].broadcast_to([128, 2, n, M])
                p1 = PAs[:, : 2 * n * M].rearrange("p (j c) -> p j c", j=2)
                p2 = PBs[:, : 2 * n * M].rearrange("p (j c) -> p j c", j=2)
                nc.vector.tensor_mul(p1, car4, inv_pair(C, k, n))
                nc.vector.tensor_mul(p2, cai4, inv_pair(C, k, n))
                fre = FRe[k % 2][:, : n * M]
                fim = FIm[k % 2][:, : n * M]
                # fre = cr*ir + ci*ii, fim = ci*ir - cr*ii
                nc.vector.tensor_add(fre, PAs[:, : n * M], PBs[:, n * M : 2 * n * M])
                nc.vector.tensor_sub(fim, PBs[:, : n * M], PAs[:, n * M : 2 * n * M])

            def elim_group(C, eng, sa, sb, k, j0, w, i0, nr):
                """Eliminate col k from planes [j0, j0+w), rows [i0, i0+nr).

                H[i,j] -= F_i * H[k,j]:  hr -= fre*Br - fim*Bi,
                                         hi -= fre*Bi + fim*Br.
                """
                HRe_, HIm_ = C["HRe"], C["HIm"]

                def wrows(T):
                    base = _off(j0, i0)
                    return T[:, base : base + w * A * M].rearrange(
                        "p (w c) -> p w c", w=w
                    )[:, :, : nr * M]

                def wrow_b(T):
                    base = _off(j0, k)
                    v = T[:, base : base + w * A * M].rearrange(
                        "p (w c) -> p w c", w=w
                    )[:, :, :M]
                    return v[:, :, None, :].broadcast_to([128, w, nr, M])

                def fw(Ft):
                    o = (i0 - k - 1) * M
                    v = Ft[:, o : o + nr * M].rearrange("p (r c) -> p r c", r=nr)
                    return v[:, None, :, :].broadcast_to([128, w, nr, M])

                hr, hi = wrows(HRe_), wrows(HIm_)
                Br, Bi = wrow_b(HRe_), wrow_b(HIm_)
                frew, fimw = fw(FRe[k % 2]), fw(FIm[k % 2])
                sz = w * nr * M
                A4 = sa[:, :sz].rearrange("p (w r c) -> p w r c", w=w, r=nr)
                A3 = sa[:, :sz].rearrange("p (w c) -> p w c", w=w)
                B4 = sb[:, :sz].rearrange("p (w r c) -> p w r c", w=w, r=nr)
                B3 = sb[:, :sz].rearrange("p (w c) -> p w c", w=w)
                eng.tensor_mul(A4, frew, Br)
                eng.tensor_mul(B4, frew, Bi)
                eng.tensor_sub(hr, hr, A3)
                eng.tensor_sub(hi, hi, B3)
                eng.tensor_mul(A4, fimw, Bi)
                eng.tensor_mul(B4, fimw, Br)
                eng.tensor_add(hr, hr, A3)
                eng.tensor_sub(hi, hi, B3)

            def emit_groups(C, eng, sa, sb, k, planes, i0, nr, wmax):
                js = list(planes)
                while js:
                    w = 1
                    while w < wmax and w < len(js) and js[w] == js[0] + w:
                        w += 1
                    elim_group(C, eng, sa, sb, k, js[0], w, i0, nr)
                    js = js[w:]

            def emit_prep0(C):
                pivot_chain(C, 0)
                factors(C, 0)

            def fwd_step(C, k, dve=True, pool=True, bias=0.0):
                """Elimination step k.  Emits the Pool share and/or the DVE
                share (incl. next step's pivot+factors pipelining)."""
                n = A - 1 - k
                npl, rs = plan_fwd(k, bias)
                j_b = 8 - npl
                if dve:
                    # pivot plane k+1 first, then next step's prep
                    elim_group(C, nc.vector, PAs, PBs, k, k + 1, 1, k + 1, n)
                    pivot_chain(C, k + 1)
                    if k + 1 < A - 1:
                        factors(C, k + 1)
                if pool:
                    pool_planes = list(range(j_b + 1, 9))
                    if pool_planes:
                        emit_groups(
                            C, nc.gpsimd, PPa, PPb, k, pool_planes, k + 1, n, W_P
                        )
                    if rs > 0:
                        elim_group(C, nc.gpsimd, PPa, PPb, k, j_b, 1, 8 - rs, rs)
                if dve:
                    dve_full = [j for j in range(k + 2, j_b + (0 if rs else 1))]
                    if dve_full:
                        emit_groups(
                            C, nc.vector, PAs, PBs, k, dve_full, k + 1, n, W_D
                        )
                    if rs > 0 and (n - rs) > 0:
                        elim_group(C, nc.vector, PAs, PBs, k, j_b, 1, k + 1, n - rs)

            def bwd_step(C, k, use_pool):
                """Jordan back step k: x_k = y_k*conj(p)/|p|^2, store it,
                then clear column k above the diagonal."""
                HRe_, HIm_ = C["HRe"], C["HIm"]
                yr = row(HRe_, 8, k)
                yi = row(HIm_, 8, k)
                p1 = PAs[:, : 2 * M].rearrange("p (j c) -> p j c", j=2)
                p2 = PBs[:, : 2 * M].rearrange("p (j c) -> p j c", j=2)
                yr2 = yr[:, None, :].broadcast_to([128, 2, M])
                yi2 = yi[:, None, :].broadcast_to([128, 2, M])
                nc.vector.tensor_mul(p1, yr2, inv_pair(C, k))
                nc.vector.tensor_mul(p2, yi2, inv_pair(C, k))
                # xr = yr*ir + yi*ii, xi = yi*ir - yr*ii
                nc.vector.tensor_add(yr, PAs[:, :M], PBs[:, M : 2 * M])
                nc.vector.tensor_sub(yi, PBs[:, :M], PAs[:, M : 2 * M])
                # x_k is final now -- store it while the back pass continues.
                so = stgo.tile([M, 2 * FS], F32, tag="so")
                so3 = so.rearrange("p (f c) -> p f c", c=2)
                for comp in range(2):
                    po = pso_pool.tile([M, FS], F32, tag="pso")
                    nc.tensor.transpose(
                        po, row(C[("HRe", "HIm")[comp]], 8, k), ident[:128, :128]
                    )
                    nc.scalar.copy(so3[:, :, comp], po)
                nc.sync.dma_start(out[k, :, C["b0"] : C["b0"] + NB], so)
                if k == 0:
                    return
                m = plan_bwd(k) if use_pool else 0

                def yupd(eng, sa, sb, r0, nr):
                    # y_i -= H[i,k]*x_k for rows [r0, r0+nr)
                    cr = rows3(HRe_, k, r0, nr)
                    ci_ = rows3(HIm_, k, r0, nr)
                    xrB = bc(yr, nr)
                    xiB = bc(yi, nr)
                    qa = sa[:, : nr * M].rearrange("p (r c) -> p r c", r=nr)
                    qc = sa[:, 7 * M : (7 + nr) * M].rearrange(
                        "p (r c) -> p r c", r=nr
                    )
                    qb = sb[:, : nr * M].rearrange("p (r c) -> p r c", r=nr)
                    qd = sb[:, 7 * M : (7 + nr) * M].rearrange(
                        "p (r c) -> p r c", r=nr
                    )
                    ytr = rows3(HRe_, 8, r0, nr)
                    yti = rows3(HIm_, 8, r0, nr)
                    eng.tensor_mul(qa, cr, xrB)
                    eng.tensor_mul(qc, cr, xiB)
                    eng.tensor_sub(ytr, ytr, qa)
                    eng.tensor_sub(yti, yti, qc)
                    eng.tensor_mul(qb, ci_, xiB)
                    eng.tensor_mul(qd, ci_, xrB)
                    eng.tensor_add(ytr, ytr, qb)
                    eng.tensor_sub(yti, yti, qd)

                if m > 0:
                    yupd(nc.gpsimd, PPa, PPb, 0, m)
                if k - m > 0:
                    yupd(nc.vector, PAs, PBs, m, k - m)

            # ---------------- emission schedule ----------------
            c0 = make_chunk(0)
            c1 = make_chunk(1)

            emit_load(c0)
            emit_prep0(c0)
            for k in range(A - 1):
                fwd_step(c0, k)
            emit_load(c1)
            # chunk 1 prep + Pool's step-0 share start while DVE runs
            # chunk 0's back pass, interleaved with chunk 1's DVE forward
            # steps; the planner bias hands Pool the slack.
            emit_prep0(c1)
            fwd_step(c1, 0, dve=False, pool=True, bias=bwd_step_cost(7))
            bwd_step(c0, 7, use_pool=False)
            fwd_step(c1, 0, dve=True, pool=False, bias=bwd_step_cost(7))
            for k in range(1, A - 1):
                j = 7 - k
                bwd_step(c0, j, use_pool=False)
                fwd_step(c1, k, bias=bwd_step_cost(j))
            bwd_step(c0, 0, use_pool=False)
            for k in range(A - 1, -1, -1):
                bwd_step(c1, k, use_pool=True)

    nc.finalize()
    return nc


_NC_CACHE = None


def _get_nc():
    global _NC_CACHE
    if _NC_CACHE is None:
        _NC_CACHE = _build()
    return _NC_CACHE


def _prep_core(y_re, y_im, h_re, h_im, c):
    """Host-side shard prep for core c: f-slice + block-diagonal extraction."""
    fsl = slice(c * FS, (c + 1) * FS)
    ue = np.arange(U)
    maps = {}
    for name, h in (("hd_re", h_re), ("hd_im", h_im)):
        h6 = h[:, 0, :, :, :, :, fsl].reshape(B, U, A, U, A, S, FS)
        hd = h6[:, ue, :, ue]              # [u, b, i, j, s, f]
        maps[name] = np.ascontiguousarray(
            hd.transpose(3, 0, 1, 4, 2, 5), dtype=np.float32
        )                                   # [j, u, b, s, i, f]
    for name, y in (("yd_re", y_re), ("yd_im", y_im)):
        y5 = y[:, 0, :, :, fsl].reshape(B, U, A, S, FS)   # [b, u, i, s, f]
        maps[name] = np.ascontiguousarray(
            y5.transpose(1, 0, 3, 2, 4), dtype=np.float32
        )                                   # [u, b, s, i, f]
    return maps


def kernel(y_re, y_im, h_re, h_im, **_ignored):
    global LAST_RESULTS
    y_re = np.asarray(y_re, dtype=np.float32)
    y_im = np.asarray(y_im, dtype=np.float32)
    h_re = np.asarray(h_re, dtype=np.float32)
    h_im = np.asarray(h_im, dtype=np.float32)

    nc = _get_nc()
    in_maps = [_prep_core(y_re, y_im, h_re, h_im, c) for c in range(NCORES)]
    trace = bool(int(os.environ.get("BD_TRACE", "0")))
    res = run_bass_kernel_spmd(
        nc, in_maps, core_ids=list(range(NCORES)), trace=trace
    )
    LAST_RESULTS = res
    outs = []
    for r in res.results:
        o = r["out"]                              # [i, u, b, s, f, c]
        o = o.transpose(2, 1, 0, 3, 4, 5)         # [b, u, i, s, f, c]
        outs.append(o.reshape(B, NR, S, FS, 2))
    full = np.concatenate(outs, axis=3)           # [B, NR, S, F, 2]
    return np.ascontiguousarray(full[:, None])    # [B, 1, NR, S, F, 2]
